# revision 16
# baseline (speedup 1.0000x reference)
"""Trainium2 Bass kernel for 16-group CustomGroupedConv2D.

Problem (hardcoded): x (16, 256, 128, 128) f32, W (512, 16, 3, 3) f32,
b (512,) f32, groups=16, 3x3, stride 1, pad 1 -> y (16, 512, 128, 128) f32.

Sharding: data-parallel over batch, 2 images per core on 8 cores; each core
writes its own output slice (no collectives).

Compute: the 128x128 PE array is a 4x2 grid of 32x64 sub-arrays via
tile_position; each holds a block-diagonal group PAIR (K=32: two groups' 16
cins; M=64: their couts). The 9 conv taps are 9 full-width accumulating
passes (PSUM start/stop) whose shifted windows are pure AP offsets into a
zero-padded 132-pitch SBUF image buffer.

Phased I/O (measured: HBM->SBUF load DMAs overlapping PE execution tax the
PE ~2x, while SBUF->HBM stores are free, and serializing the loads costs
less than the overlap tax):
- x is cast bf16 on the host; per half, two fully contiguous 2.13MB loads
  land in a flat staging buffer. The loads are triggered from the Act/SP
  queues, so engine program order gates them behind the previous half's
  evacuations/stores: they transfer while the PE is between halves instead
  of taxing it mid-compute.
- VectorE re-lays staging into the padded 132-pitch buffer (pads and halo
  slabs are zeroed once at startup and never rewritten).
- y is written bf16 (host upcasts; ~1e-3 extra rounding, tolerance is
  2e-2) with stores batched 4 windows per DMA: 128 stores of 4KB/line.

Bias is fused into the PSUM->SBUF evacuation (ScalarE even banks, VectorE
odd) along with the f32->bf16 cast. Each window's 4 PSUM banks are shared
by the (r, s=0/1) PE tiles (partitions 0:64/64:128); hardware clears
has_written per partition-range on each tile's first start=True matmul.
"""

import numpy as np

N_CORES = 8
N, CIN, H, W_IMG = 16, 256, 128, 128
COUT, KH, KW = 512, 3, 3
GROUPS = 16
CPG = CIN // GROUPS  # 16 cins per group
MPG = COUT // GROUPS  # 32 couts per group
N_PER_CORE = N // N_CORES  # 2 images
SLABS = 66  # padded row slabs per half (65 rows + 1 halo/zero)
WPAD = 132  # padded row pitch (col 0 and 129 are the zero pads)
WIN_ROWS = 4  # output rows per window (N = 4*128 = 512)
WINS = 16  # windows per half
SW_WINS = 4  # windows per store super-window (16 output rows per store DMA)

_CACHE = {}


def _bank_groups(r):
    """Groups whose couts live in psum bank r, in col-strip order."""
    return [2 * r, 2 * r + 1, 2 * r + 8, 2 * r + 9]


def _prep_weights(W):
    # W: (COUT, CPG, 3, 3) -> W_prep [128, 9, 2, 64] bf16, block-diagonal
    # group pairs: partition 32r+i, tap t=3*dy+dx, slot s holds the [32, 64]
    # lhsT for the pair (g0, g1) = (8s+2r, 8s+2r+1).
    import ml_dtypes

    Wp = np.zeros((128, KH * KW, 2, 2 * MPG), np.float32)
    for r in range(4):
        for s in range(2):
            for half in range(2):
                g = 8 * s + 2 * r + half
                blk = W[g * MPG : (g + 1) * MPG]  # (32, 16, 3, 3)
                lhsT = np.transpose(blk, (1, 2, 3, 0)).reshape(CPG, KH * KW, MPG)
                Wp[
                    32 * r + 16 * half : 32 * r + 16 * (half + 1),
                    :,
                    s,
                    MPG * half : MPG * (half + 1),
                ] = lhsT
    return Wp.astype(ml_dtypes.bfloat16)


def _prep_bias(b):
    # b: (COUT,) -> b_prep [128, 4]; partition 32j+m, col r = b[G(r,j)*32+m]
    br = b.reshape(GROUPS, MPG)
    bp = np.zeros((128, 4), np.float32)
    for r in range(4):
        for j, g in enumerate(_bank_groups(r)):
            bp[32 * j : 32 * j + 32, r] = br[g]
    return bp


def _build_program(reps=1, mode="full"):
    # mode: component letters + option letters (timing decomposition / A-B
    # probes). Components: L loads, C relayout copies, M matmuls, E evac,
    # S stores. Options: g loads via gpsimd queue, a 4B-aligned padded
    # layout (data at cols 2:130), b bf16 psum accumulation.
    # "full" == "LCMES".
    import concourse.bacc as bacc
    import concourse.mybir as mybir
    import concourse.tile as tile
    from contextlib import nullcontext

    if mode == "full":
        mode = "LCMES"
    L, C, M, E, S = (k in mode for k in "LCMES")
    opt_g, opt_a, opt_b = (k in mode for k in "gab")
    col0 = 2 if opt_a else 1  # first data column in the padded buffer

    f32 = mybir.dt.float32
    bf16 = mybir.dt.bfloat16
    ACT_IDENT = mybir.ActivationFunctionType.Identity

    nc = bacc.Bacc(
        "TRN2", target_bir_lowering=False, debug=False, num_devices=N_CORES
    )
    x_d = nc.dram_tensor("x", [N_PER_CORE, CIN, H, W_IMG], bf16, kind="ExternalInput")
    w_d = nc.dram_tensor("wp", [128, 9, 2, 2 * MPG], bf16, kind="ExternalInput")
    b_d = nc.dram_tensor("bp", [128, 4], f32, kind="ExternalInput")
    y_d = nc.dram_tensor(
        "y", [N_PER_CORE, COUT, H, W_IMG], bf16, kind="ExternalOutput"
    )

    with tile.TileContext(nc) as tc:
        with (
            tc.tile_pool(name="wpool", bufs=1) as wpool,
            tc.tile_pool(name="xstgp", bufs=2) as xstgp,
            tc.tile_pool(name="ppool", bufs=8, space="PSUM") as ppool,
            tc.tile_pool(name="spool", bufs=2) as spool,
        ):
            w_sb = wpool.tile([128, 9, 2, 2 * MPG], bf16, tag="w")
            nc.sync.dma_start(w_sb[:], w_d[:])
            b_sb = wpool.tile([128, 4], f32, tag="b")
            nc.sync.dma_start(b_sb[:], b_d[:])
            # one padded image buffer per half-parity; pads and halo slabs
            # are zeroed here once and never rewritten
            xv = [
                wpool.tile([128, 2, SLABS, WPAD], bf16, tag=f"xv{p}", name="xv")
                for p in range(2)
            ]
            for p in range(2):
                nc.gpsimd.memset(xv[p][:], 0.0)
            static_stg = None
            if S and not E:
                static_stg = wpool.tile(
                    [128, SW_WINS * WIN_ROWS, W_IMG], bf16, tag="sstg"
                )
                nc.gpsimd.memset(static_stg[:], 0.5)

            # reps>1 repeats the whole computation on-device (timing only)
            rep_ctx = tc.For_i(0, reps, 1) if reps > 1 else nullcontext()
            with rep_ctx:
              for n in range(N_PER_CORE):
                  for hf in range(2):
                      xb = xv[hf]
                      if L:
                          xstg = xstgp.tile(
                              [128, 2, 65, W_IMG], bf16, tag="xstg"
                          )
                          row0 = 0 if hf == 0 else 63
                          ld = nc.gpsimd if opt_g else nc.scalar
                          ld2 = nc.gpsimd if opt_g else nc.sync
                          ld.dma_start(
                              xstg[:, 0], x_d[n, 0:128, row0 : row0 + 65, :]
                          )
                          ld2.dma_start(
                              xstg[:, 1], x_d[n, 128:256, row0 : row0 + 65, :]
                          )
                          # VectorE re-layout into the padded buffer
                          # (hf0: rows 0..64 -> slabs 1..65; hf1: rows
                          # 63..127 -> slabs 0..64)
                          if C:
                              slab0 = 1 if hf == 0 else 0
                              for s in range(2):
                                  nc.vector.tensor_copy(
                                      xb[
                                          :,
                                          s,
                                          slab0 : slab0 + 65,
                                          col0 : col0 + W_IMG,
                                      ],
                                      xstg[:, s],
                                  )
                      if not (M or E or S):
                          continue
                      for sw in range(WINS // SW_WINS):
                          stg = [
                              spool.tile(
                                  [128, SW_WINS * WIN_ROWS, W_IMG],
                                  bf16,
                                  tag=f"stg{r}",
                                  name="stg",
                              )
                              for r in range(4)
                          ]
                          out_row0 = 64 * hf + SW_WINS * WIN_ROWS * sw
                          if S and not E:
                              for r in range(4):
                                  for s, co0 in ((0, 64 * r), (1, 256 + 64 * r)):
                                      nc.sync.dma_start(
                                          y_d[
                                              n,
                                              co0 : co0 + 64,
                                              out_row0 : out_row0
                                              + SW_WINS * WIN_ROWS,
                                              :,
                                          ],
                                          static_stg[64 * s : 64 * s + 64, :, :],
                                      )
                              continue
                          for wl in range(SW_WINS):
                              w = SW_WINS * sw + wl
                              ps = [
                                  ppool.tile(
                                      [128, WIN_ROWS, W_IMG],
                                      bf16 if opt_b else f32,
                                      tag="ps",
                                      name="ps",
                                  )
                                  for _ in range(4)
                              ]
                              if M:
                                  for t in range(9):
                                      dy, dx = t // 3, t % 3
                                      for r in range(4):
                                          for s in range(2):
                                              nc.tensor.matmul(
                                                  ps[r][64 * s : 64 * s + 64, :, :],
                                                  w_sb[32 * r : 32 * r + 32, t, s, :],
                                                  xb[
                                                      32 * r : 32 * r + 32,
                                                      s,
                                                      WIN_ROWS * w + dy : WIN_ROWS * w
                                                      + dy
                                                      + WIN_ROWS,
                                                      col0 - 1 + dx : col0 - 1 + dx + W_IMG,
                                                  ],
                                                  start=(t == 0),
                                                  stop=(t == 8),
                                                  tile_position=(32 * r, 64 * s),
                                                  skip_group_check=True,
                                              )
                              if not E:
                                  continue
                              # evacuate into this window's quarter of the
                              # super-window staging tiles, fusing bias and
                              # the f32->bf16 cast; ScalarE/VectorE split
                              for r in range(4):
                                  dst = stg[r][
                                      :, WIN_ROWS * wl : WIN_ROWS * (wl + 1), :
                                  ]
                                  if r % 2 == 0:
                                      nc.scalar.activation(
                                          dst,
                                          ps[r][:],
                                          ACT_IDENT,
                                          bias=b_sb[:, r : r + 1],
                                      )
                                  else:
                                      nc.vector.tensor_scalar_add(
                                          dst,
                                          ps[r][:],
                                          b_sb[:, r : r + 1],
                                      )
                          # one store DMA per (r, s) per super-window:
                          # 16 output rows, 4 KB/partition-line
                          if not S:
                              continue
                          for r in range(4):
                              # couts: partitions 0:64 -> 64r..64r+64 (s=0),
                              # partitions 64:128 -> 256+64r.. (s=1)
                              for s, co0 in ((0, 64 * r), (1, 256 + 64 * r)):
                                  nc.sync.dma_start(
                                      y_d[
                                          n,
                                          co0 : co0 + 64,
                                          out_row0 : out_row0 + SW_WINS * WIN_ROWS,
                                          :,
                                      ],
                                      stg[r][64 * s : 64 * s + 64, :, :],
                                  )

    nc.compile()
    return nc


def _build_v2(reps=1, mode="full"):
    # v2: merged 2 MB store DMAs via multi-dim DRAM APs, 4B-aligned padded
    # layout. fp32 PSUM (bass requires fp32 matmul output), N=512 windows.
    # mode: component letters L loads, C copies, M matmuls, E evac,
    # S stores; options: g loads on the gpsimd queue, f flat-rhs probe
    # (timing only, wrong numerics), l standalone full-array ldweights +
    # non-self-loading matmuls. "full" == "LCMES".
    import concourse.bacc as bacc
    import concourse.mybir as mybir
    import concourse.tile as tile
    from contextlib import nullcontext

    if mode == "full":
        mode = "LCMES"
    L, C, M, E, S = (k in mode for k in "LCMES")
    opt_g = "g" in mode
    opt_f = "f" in mode
    opt_l = "l" in mode

    COL0 = 2  # first data column in the padded buffer (4B-aligned)

    f32 = mybir.dt.float32
    bf16 = mybir.dt.bfloat16
    ACT_IDENT = mybir.ActivationFunctionType.Identity

    nc = bacc.Bacc(
        "TRN2", target_bir_lowering=False, debug=False, num_devices=N_CORES
    )
    x_d = nc.dram_tensor("x", [N_PER_CORE, CIN, H, W_IMG], bf16, kind="ExternalInput")
    w_d = nc.dram_tensor("wp", [128, 9, 2, 2 * MPG], bf16, kind="ExternalInput")
    b_d = nc.dram_tensor("bp", [128, 4], f32, kind="ExternalInput")
    y_d = nc.dram_tensor(
        "y", [N_PER_CORE, COUT, H, W_IMG], bf16, kind="ExternalOutput"
    )
    # [n, s, co, r, h, w] view: store dst iterates (s, co) partitions then
    # (r, h, w) free dims to match stg partition (64s+co) x free (r, h, w)
    y_v = y_d.reshape([N_PER_CORE, 2, 4, 64, H, W_IMG]).rearrange(
        "n s r c h w -> n s c r h w"
    )

    with tile.TileContext(nc) as tc:
        with (
            tc.tile_pool(name="wpool", bufs=1) as wpool,
            tc.tile_pool(name="xstgp", bufs=2) as xstgp,
            tc.tile_pool(name="ppool", bufs=8, space="PSUM") as ppool,
            tc.tile_pool(name="spool", bufs=2) as spool,
        ):
            w_sb = wpool.tile([128, 9, 2, 2 * MPG], bf16, tag="w")
            nc.sync.dma_start(w_sb[:], w_d[:])
            # full-array block-diagonal tap weights for the 'l' probe:
            # partition 32r+i, tap t, col 64s+m = w_sb[32r+i, t, s, m]
            wl_sb = None
            if opt_l:
                wl_sb = wpool.tile([128, 9, 128], bf16, tag="wl")
                for s in range(2):
                    nc.vector.tensor_copy(
                        wl_sb[:, :, 64 * s : 64 * s + 64], w_sb[:, :, s, :]
                    )
            b_sb = wpool.tile([128, 4], f32, tag="b")
            nc.sync.dma_start(b_sb[:], b_d[:])
            xv = [
                wpool.tile([128, 2, SLABS, WPAD], bf16, tag=f"xv{p}", name="xv")
                for p in range(2)
            ]
            for p in range(2):
                nc.gpsimd.memset(xv[p][:], 0.0)
            static_stg = None
            if S and not E:
                static_stg = wpool.tile(
                    [128, 4, SW_WINS * WIN_ROWS, W_IMG], bf16, tag="sstg"
                )
                nc.gpsimd.memset(static_stg[:], 0.5)

            rep_ctx = tc.For_i(0, reps, 1) if reps > 1 else nullcontext()
            with rep_ctx:
              for n in range(N_PER_CORE):
                  for hf in range(2):
                      xb = xv[hf]
                      if L:
                          xstg = xstgp.tile(
                              [128, 2, 65, W_IMG], bf16, tag="xstg"
                          )
                          row0 = 0 if hf == 0 else 63
                          ld = nc.gpsimd if opt_g else nc.scalar
                          ld2 = nc.gpsimd if opt_g else nc.sync
                          ld.dma_start(
                              xstg[:, 0], x_d[n, 0:128, row0 : row0 + 65, :]
                          )
                          ld2.dma_start(
                              xstg[:, 1], x_d[n, 128:256, row0 : row0 + 65, :]
                          )
                          if C:
                              slab0 = 1 if hf == 0 else 0
                              for s in range(2):
                                  nc.vector.tensor_copy(
                                      xb[
                                          :,
                                          s,
                                          slab0 : slab0 + 65,
                                          COL0 : COL0 + W_IMG,
                                      ],
                                      xstg[:, s],
                                  )
                      if not (M or E or S):
                          continue
                      for sw in range(WINS // SW_WINS):
                          stg = spool.tile(
                              [128, 4, SW_WINS * WIN_ROWS, W_IMG],
                              bf16,
                              tag="stg",
                              name="stg",
                          )
                          out_row0 = 64 * hf + SW_WINS * WIN_ROWS * sw
                          if S and not E:
                              for r in range(4):
                                  for s, co0 in ((0, 64 * r), (1, 256 + 64 * r)):
                                      nc.sync.dma_start(
                                          y_d[
                                              n,
                                              co0 : co0 + 64,
                                              out_row0 : out_row0
                                              + SW_WINS * WIN_ROWS,
                                              :,
                                          ],
                                          static_stg[64 * s : 64 * s + 64, r],
                                      )
                              continue
                          for wl in range(SW_WINS):
                              w = SW_WINS * sw + wl
                              ps = [
                                  ppool.tile(
                                      [128, WIN_ROWS, W_IMG], f32, tag="ps",
                                      name="ps",
                                  )
                                  for _ in range(4)
                              ]
                              if M:
                                  for t in range(9):
                                      dy, dx = t // 3, t % 3
                                      if opt_l:
                                          nc.tensor.ldweights(wl_sb[:, t, :])
                                      for r in range(4):
                                          for s in range(2):
                                              if opt_f:
                                                  rhs = xb[
                                                      32 * r : 32 * r + 32,
                                                      s,
                                                      WIN_ROWS * w : WIN_ROWS * w
                                                      + 4,
                                                      0:W_IMG,
                                                  ]
                                              else:
                                                  rhs = xb[
                                                      32 * r : 32 * r + 32,
                                                      s,
                                                      WIN_ROWS * w + dy : WIN_ROWS
                                                      * w
                                                      + dy
                                                      + WIN_ROWS,
                                                      COL0 - 1 + dx : COL0
                                                      - 1
                                                      + dx
                                                      + W_IMG,
                                                  ]
                                              mm = nc.tensor.matmul(
                                                  ps[r][64 * s : 64 * s + 64, :, :],
                                                  w_sb[32 * r : 32 * r + 32, t, s, :],
                                                  rhs,
                                                  start=(t == 0),
                                                  stop=(t == 8),
                                                  tile_position=(32 * r, 64 * s),
                                                  skip_group_check=True,
                                              )
                                              if opt_l:
                                                  mm.ldweights = False
                              if not E:
                                  continue
                              for r in range(4):
                                  dst = stg[
                                      :,
                                      r,
                                      WIN_ROWS * wl : WIN_ROWS * (wl + 1),
                                      :,
                                  ]
                                  if r % 2 == 0:
                                      nc.scalar.activation(
                                          dst,
                                          ps[r][:],
                                          ACT_IDENT,
                                          bias=b_sb[:, r : r + 1],
                                      )
                                  else:
                                      nc.vector.tensor_scalar_add(
                                          dst,
                                          ps[r][:],
                                          b_sb[:, r : r + 1],
                                      )
                          if not S:
                              continue
                          for r in range(4):
                              for s, co0 in ((0, 64 * r), (1, 256 + 64 * r)):
                                  nc.sync.dma_start(
                                      y_d[
                                          n,
                                          co0 : co0 + 64,
                                          out_row0 : out_row0
                                          + SW_WINS * WIN_ROWS,
                                          :,
                                      ],
                                      stg[64 * s : 64 * s + 64, r],
                                  )

    nc.compile()
    return nc


def _build_v3(reps=1, mode="full"):
    # v3 = v2 + software-pipelined x loads: half i+1's staging loads are
    # issued (HWDGE sync/scalar queues) at the top of half i's body, so the
    # engine-queue gating starts the transfer right at half i's compute
    # start; the DVE relayout for i+1 runs at the end of half i, behind its
    # last evacs, overlapping the PE's final windows. The last half
    # prefetches half 0 again (steady-state across on-device reps).
    # mode: L loads, C copies, M matmuls, E evac, S stores; "full"="LCMES".
    import concourse.bacc as bacc
    import concourse.mybir as mybir
    import concourse.tile as tile
    from contextlib import nullcontext

    if mode == "full":
        mode = "LCMES"
    L, C, M, E, S = (k in mode for k in "LCMES")

    COL0 = 2  # first data column in the padded buffer (4B-aligned)

    f32 = mybir.dt.float32
    bf16 = mybir.dt.bfloat16
    ACT_IDENT = mybir.ActivationFunctionType.Identity

    nc = bacc.Bacc(
        "TRN2", target_bir_lowering=False, debug=False, num_devices=N_CORES
    )
    x_d = nc.dram_tensor("x", [N_PER_CORE, CIN, H, W_IMG], bf16, kind="ExternalInput")
    w_d = nc.dram_tensor("wp", [128, 9, 2, 2 * MPG], bf16, kind="ExternalInput")
    b_d = nc.dram_tensor("bp", [128, 4], f32, kind="ExternalInput")
    y_d = nc.dram_tensor(
        "y", [N_PER_CORE, COUT, H, W_IMG], bf16, kind="ExternalOutput"
    )

    halves = [(n, hf) for n in range(N_PER_CORE) for hf in range(2)]

    with tile.TileContext(nc) as tc:
        with (
            tc.tile_pool(name="wpool", bufs=1) as wpool,
            tc.tile_pool(name="xstgp", bufs=2) as xstgp,
            tc.tile_pool(name="ppool", bufs=8, space="PSUM") as ppool,
            tc.tile_pool(name="spool", bufs=2) as spool,
        ):
            w_sb = wpool.tile([128, 9, 2, 2 * MPG], bf16, tag="w")
            nc.sync.dma_start(w_sb[:], w_d[:])
            b_sb = wpool.tile([128, 4], f32, tag="b")
            nc.sync.dma_start(b_sb[:], b_d[:])
            xv = [
                wpool.tile([128, 2, SLABS, WPAD], bf16, tag=f"xv{p}", name="xv")
                for p in range(2)
            ]
            for p in range(2):
                nc.gpsimd.memset(xv[p][:], 0.0)
            static_stg = None
            if S and not E:
                static_stg = wpool.tile(
                    [128, 4, SW_WINS * WIN_ROWS, W_IMG], bf16, tag="sstg"
                )
                nc.gpsimd.memset(static_stg[:], 0.5)

            def issue_load(i):
                # stage half halves[i % 4] into the next xstg pool buffer
                n_, hf_ = halves[i % len(halves)]
                xstg = xstgp.tile([128, 2, 65, W_IMG], bf16, tag="xstg")
                row0 = 0 if hf_ == 0 else 63
                nc.scalar.dma_start(
                    xstg[:, 0], x_d[n_, 0:128, row0 : row0 + 65, :]
                )
                nc.scalar.dma_start(
                    xstg[:, 1], x_d[n_, 128:256, row0 : row0 + 65, :]
                )
                return xstg

            def relayout(i, xstg):
                # DVE copy of staged half i into its padded parity buffer
                _, hf_ = halves[i % len(halves)]
                slab0 = 1 if hf_ == 0 else 0
                for s in range(2):
                    nc.vector.tensor_copy(
                        xv[hf_][:, s, slab0 : slab0 + 65, COL0 : COL0 + W_IMG],
                        xstg[:, s],
                    )

            # prologue: stage + relay half 0 (runs once, outside the reps)
            if L:
                stg0 = issue_load(0)
                if C:
                    relayout(0, stg0)

            rep_ctx = tc.For_i(0, reps, 1) if reps > 1 else nullcontext()
            with rep_ctx:
              for i, (n, hf) in enumerate(halves):
                  xb = xv[hf]
                  if L:
                      xstg_next = issue_load(i + 1)
                  if M or E or S:
                      for sw in range(WINS // SW_WINS):
                          stg = spool.tile(
                              [128, 4, SW_WINS * WIN_ROWS, W_IMG],
                              bf16,
                              tag="stg",
                              name="stg",
                          )
                          out_row0 = 64 * hf + SW_WINS * WIN_ROWS * sw
                          if S and not E:
                              for r in range(4):
                                  for s, co0 in ((0, 64 * r), (1, 256 + 64 * r)):
                                      nc.sync.dma_start(
                                          y_d[
                                              n,
                                              co0 : co0 + 64,
                                              out_row0 : out_row0
                                              + SW_WINS * WIN_ROWS,
                                              :,
                                          ],
                                          static_stg[64 * s : 64 * s + 64, r],
                                      )
                              continue
                          for wl in range(SW_WINS):
                              w = SW_WINS * sw + wl
                              ps = [
                                  ppool.tile(
                                      [128, WIN_ROWS, W_IMG], f32, tag="ps",
                                      name="ps",
                                  )
                                  for _ in range(4)
                              ]
                              if M:
                                  for t in range(9):
                                      dy, dx = t // 3, t % 3
                                      for r in range(4):
                                          for s in range(2):
                                              nc.tensor.matmul(
                                                  ps[r][64 * s : 64 * s + 64, :, :],
                                                  w_sb[32 * r : 32 * r + 32, t, s, :],
                                                  xb[
                                                      32 * r : 32 * r + 32,
                                                      s,
                                                      WIN_ROWS * w + dy : WIN_ROWS
                                                      * w
                                                      + dy
                                                      + WIN_ROWS,
                                                      COL0 - 1 + dx : COL0
                                                      - 1
                                                      + dx
                                                      + W_IMG,
                                                  ],
                                                  start=(t == 0),
                                                  stop=(t == 8),
                                                  tile_position=(32 * r, 64 * s),
                                                  skip_group_check=True,
                                              )
                              if E:
                                  for r in range(4):
                                      dst = stg[
                                          :,
                                          r,
                                          WIN_ROWS * wl : WIN_ROWS * (wl + 1),
                                          :,
                                      ]
                                      if r % 2 == 0:
                                          nc.scalar.activation(
                                              dst,
                                              ps[r][:],
                                              ACT_IDENT,
                                              bias=b_sb[:, r : r + 1],
                                          )
                                      else:
                                          nc.vector.tensor_scalar_add(
                                              dst,
                                              ps[r][:],
                                              b_sb[:, r : r + 1],
                                          )
                          if E and S:
                              for r in range(4):
                                  for s, co0 in ((0, 64 * r), (1, 256 + 64 * r)):
                                      nc.sync.dma_start(
                                          y_d[
                                              n,
                                              co0 : co0 + 64,
                                              out_row0 : out_row0
                                              + SW_WINS * WIN_ROWS,
                                              :,
                                          ],
                                          stg[64 * s : 64 * s + 64, r],
                                      )
                  if L and C:
                      relayout(i + 1, xstg_next)

    nc.compile()
    return nc


def _build_v5(reps=1, mode="full"):
    # v5: no column padding. Loads land directly in the compute buffer as
    # fully contiguous DMAs (no DVE relayout). Each tap is a ragged matmul:
    # dx=0 writes out cols 1:128, dx=1 cols 0:128, dx=2 cols 0:127; PSUM's
    # per-element has_written gives border columns overwrite-on-first-touch,
    # which exactly reproduces zero-padding semantics. H borders use two
    # once-zeroed buffer rows. x loads for half i+1 are issued on the ACT
    # ring at the top of half i (SP ring stays pure stores); the WAR
    # semaphore on the parity buffer delays the transfer to half i-1's PE
    # drain, overlapping half i's compute.
    # mode: L loads, M matmuls, E evac, S stores; "full" == "LMES".
    import concourse.bacc as bacc
    import concourse.mybir as mybir
    import concourse.tile as tile
    from contextlib import nullcontext

    if mode == "full":
        mode = "LMES"
    L, M, E, S = (k in mode for k in "LMES")

    XROWS = 66  # 65 loaded rows + 1 zero border row per parity

    f32 = mybir.dt.float32
    bf16 = mybir.dt.bfloat16
    ACT_IDENT = mybir.ActivationFunctionType.Identity

    # per-dx ragged extents: (rhs col0, out col0, ncols)
    DXE = {0: (0, 1, 127), 1: (0, 0, 128), 2: (1, 0, 127)}

    nc = bacc.Bacc(
        "TRN2", target_bir_lowering=False, debug=False, num_devices=N_CORES
    )
    x_d = nc.dram_tensor("x", [N_PER_CORE, CIN, H, W_IMG], bf16, kind="ExternalInput")
    w_d = nc.dram_tensor("wp", [128, 9, 2, 2 * MPG], bf16, kind="ExternalInput")
    b_d = nc.dram_tensor("bp", [128, 4], f32, kind="ExternalInput")
    y_d = nc.dram_tensor(
        "y", [N_PER_CORE, COUT, H, W_IMG], bf16, kind="ExternalOutput"
    )

    halves = [(n, hf) for n in range(N_PER_CORE) for hf in range(2)]

    with tile.TileContext(nc) as tc:
        with (
            tc.tile_pool(name="wpool", bufs=1) as wpool,
            tc.tile_pool(name="ppool", bufs=8, space="PSUM") as ppool,
            tc.tile_pool(name="spool", bufs=3) as spool,
        ):
            w_sb = wpool.tile([128, 9, 2, 2 * MPG], bf16, tag="w")
            nc.sync.dma_start(w_sb[:], w_d[:])
            b_sb = wpool.tile([128, 4], f32, tag="b")
            nc.sync.dma_start(b_sb[:], b_d[:])
            xv = [
                wpool.tile([128, 2, XROWS, W_IMG], bf16, tag=f"xv{p}", name="xv")
                for p in range(2)
            ]
            for p in range(2):
                nc.gpsimd.memset(xv[p][:], 0.0)
            static_stg = None
            if S and not E:
                static_stg = wpool.tile(
                    [128, 4, SW_WINS * WIN_ROWS, W_IMG], bf16, tag="sstg"
                )
                nc.gpsimd.memset(static_stg[:], 0.5)

            def issue_load(i):
                # direct contiguous load of half halves[i % 4] into its
                # parity buffer (rows slab0..slab0+64; border rows stay 0)
                n_, hf_ = halves[i % len(halves)]
                row0 = 0 if hf_ == 0 else 63
                slab0 = 1 if hf_ == 0 else 0
                for s in range(2):
                    nc.scalar.dma_start(
                        xv[hf_][:, s, slab0 : slab0 + 65, :],
                        x_d[n_, 128 * s : 128 * s + 128, row0 : row0 + 65, :],
                    )

            if L:
                issue_load(0)

            rep_ctx = tc.For_i(0, reps, 1) if reps > 1 else nullcontext()
            with rep_ctx:
              for i, (n, hf) in enumerate(halves):
                  xb = xv[hf]
                  if L:
                      issue_load(i + 1)
                  if not (M or E or S):
                      continue
                  for sw in range(WINS // SW_WINS):
                      stg = spool.tile(
                          [128, 4, SW_WINS * WIN_ROWS, W_IMG],
                          bf16,
                          tag="stg",
                          name="stg",
                      )
                      out_row0 = 64 * hf + SW_WINS * WIN_ROWS * sw
                      if S and not E:
                          for r in range(4):
                              for s, co0 in ((0, 64 * r), (1, 256 + 64 * r)):
                                  nc.sync.dma_start(
                                      y_d[
                                          n,
                                          co0 : co0 + 64,
                                          out_row0 : out_row0
                                          + SW_WINS * WIN_ROWS,
                                          :,
                                      ],
                                      static_stg[64 * s : 64 * s + 64, r],
                                  )
                          continue
                      for wl in range(SW_WINS):
                          w = SW_WINS * sw + wl
                          ps = [
                              ppool.tile(
                                  [128, WIN_ROWS, W_IMG], f32, tag="ps",
                                  name="ps",
                              )
                              for _ in range(4)
                          ]
                          if M:
                              for t in range(9):
                                  dy, dx = t // 3, t % 3
                                  c0, x0, ncols = DXE[dx]
                                  for r in range(4):
                                      for s in range(2):
                                          nc.tensor.matmul(
                                              ps[r][
                                                  64 * s : 64 * s + 64,
                                                  :,
                                                  x0 : x0 + ncols,
                                              ],
                                              w_sb[32 * r : 32 * r + 32, t, s, :],
                                              xb[
                                                  32 * r : 32 * r + 32,
                                                  s,
                                                  WIN_ROWS * w + dy : WIN_ROWS * w
                                                  + dy
                                                  + WIN_ROWS,
                                                  c0 : c0 + ncols,
                                              ],
                                              start=(t == 0),
                                              stop=(t == 8),
                                              tile_position=(32 * r, 64 * s),
                                              skip_group_check=True,
                                          )
                          if E:
                              for r in range(4):
                                  dst = stg[
                                      :,
                                      r,
                                      WIN_ROWS * wl : WIN_ROWS * (wl + 1),
                                      :,
                                  ]
                                  if r % 2 == 0:
                                      nc.scalar.activation(
                                          dst,
                                          ps[r][:],
                                          ACT_IDENT,
                                          bias=b_sb[:, r : r + 1],
                                      )
                                  else:
                                      nc.vector.tensor_scalar_add(
                                          dst,
                                          ps[r][:],
                                          b_sb[:, r : r + 1],
                                      )
                      if E and S:
                          for r in range(4):
                              for s, co0 in ((0, 64 * r), (1, 256 + 64 * r)):
                                  nc.sync.dma_start(
                                      y_d[
                                          n,
                                          co0 : co0 + 64,
                                          out_row0 : out_row0
                                          + SW_WINS * WIN_ROWS,
                                          :,
                                      ],
                                      stg[64 * s : 64 * s + 64, r],
                                  )

    nc.compile()
    return nc


def _get_program(reps=1):
    key = ("nc", reps)
    if key not in _CACHE:
        _CACHE[key] = _build_v5(reps)
    return _CACHE[key]


def make_in_maps(x, W, b):
    import ml_dtypes

    Wp = _prep_weights(np.asarray(W, dtype=np.float32))
    bp = _prep_bias(np.asarray(b, dtype=np.float32))
    x_bf = np.ascontiguousarray(
        np.asarray(x, dtype=np.float32).astype(ml_dtypes.bfloat16)
    )
    return [
        {
            "x": x_bf[i * N_PER_CORE : (i + 1) * N_PER_CORE],
            "wp": Wp,
            "bp": bp,
        }
        for i in range(N_CORES)
    ]


def kernel(x, W, b):
    from concourse.bass_utils import run_bass_kernel_spmd

    nc = _get_program()
    in_maps = make_in_maps(x, W, b)
    res = run_bass_kernel_spmd(nc, in_maps, list(range(N_CORES)))
    out = np.concatenate([res.results[i]["y"] for i in range(N_CORES)], axis=0)
    return out.astype(np.float32)

